# revision 8
# baseline (speedup 1.0000x reference)
"""ContrastiveLoss Trainium2 kernel.

Strategy (data-parallel over 8 NeuronCores):
  - 8 cores = 4 batches x 2 halves. Core c handles batch b=c//2, half h=c%2:
    2500 match pairs + 25000 non-match pairs.
  - Gather primitive: SWDGE vector-indirect DMA (`indirect_dma_start`) with a
    MULTI-COLUMN index AP: idx [128, k] + out [128, k*D] gathers 128*k rows
    (one 64 B row per descriptor) in a single instruction, so the ~1 us
    SWDGE fixed overhead is paid ~6 times per core instead of ~470.
    Samples are column-blocked: sample s = block j * 128 + partition p, so
    block j's indices live in idx[:, j] and its rows land in g[:, 16j:16j+16].
  - Per-sample math on DVE/ACT (overlapped with the gather stream):
      match partial  = sum((mA-mB)^2)              (DVE sub + ACT sq-accum)
      nonmatch partial = sum(relu(0.5-||nA-nB||^2)) (DVE sub, ACT square,
        DVE grouped reduce over D=16, ACT relu with fused accumulate)
  - Padding: tail samples use index 0 on both sides; a host-built {0,1} mask
    zeroes pad match diffs exactly, and a host-built additive bias pushes pad
    nonmatch distances to 1e9 so the hinge is exactly 0.
  - Partition reduction via a ones-vector TensorE matmul into PSUM.
  - Each core outputs [1,2] raw partial sums; the host combines 8x2 scalars
    and applies the 1/5000 and 1/50000 normalizations.

kernel() takes the FULL (unsharded) inputs and returns the full output tuple
(contrastive_loss_sum, match_loss_sum, nonmatch_loss_sum) like the reference.
"""

import os

import numpy as np

# Problem constants (hardcoded per task spec).
B, N, D = 4, 307200, 16
M_MATCH, M_NONMATCH = 5000, 50000
MARGIN = 0.5
NON_MATCH_WEIGHT = 1.0
NCORES = 8

P = 128
MH = M_MATCH // 2  # 2500 match samples per core
NH = M_NONMATCH // 2  # 25000 nonmatch samples per core
MBLK = (MH + P - 1) // P  # 20 match blocks (last one partial: 2500=19*128+68)
NBLK = (NH + P - 1) // P  # 196 nonmatch blocks (25000=195*128+40)
M_REM = MH - (MBLK - 1) * P  # 68 real rows in last match block
N_REM = NH - (NBLK - 1) * P  # 40 real rows in last nonmatch block
NCH = 2  # nonmatch gather chunks (pipeline SWDGE gen with DMA transfer)
BPC = NBLK // NCH  # 98 blocks per chunk
assert NCH * BPC == NBLK

LAST_EXEC_NS = None

_CACHE = {}


def _build_nc():
    import concourse.bacc as bacc
    import concourse.mybir as mybir
    import concourse.tile as tile
    from concourse import bass

    f32 = mybir.dt.float32
    i32 = mybir.dt.int32
    X = mybir.AxisListType.X
    ADD = mybir.AluOpType.add
    MULT = mybir.AluOpType.mult
    Relu = mybir.ActivationFunctionType.Relu

    nc = bacc.Bacc("TRN2", target_bir_lowering=False, debug=False)
    eA = nc.dram_tensor("eA", (N, D), f32, kind="ExternalInput")
    eB = nc.dram_tensor("eB", (N, D), f32, kind="ExternalInput")
    # all index columns in one tensor: [inA | inB | imA | imB]
    idx = nc.dram_tensor("idx", (P, 2 * NBLK + 2 * MBLK), i32, kind="ExternalInput")
    # pad handling in one tensor: [mmask (MBLK) | npad (BPC)].
    # mmask is 1.0 for real match samples else 0.0; npad adds 1e9 to pad
    # nonmatch distances (hinge -> exactly 0)
    fm = nc.dram_tensor("fm", (P, MBLK + BPC), f32, kind="ExternalInput")
    out = nc.dram_tensor("out", (P, 1 + NCH), f32, kind="ExternalOutput")

    def gather(dst_ap, src, idx_ap):
        nc.gpsimd.indirect_dma_start(
            out=dst_ap,
            out_offset=None,
            in_=src.ap(),
            in_offset=bass.IndirectOffsetOnAxis(ap=idx_ap, axis=0),
        )

    with tile.TileContext(nc) as tc:
        with (
            tc.tile_pool(name="idx", bufs=1) as idxp,
            tc.tile_pool(name="gath", bufs=2) as gp,
            tc.tile_pool(name="cmp", bufs=2) as cp,
            tc.tile_pool(name="sums", bufs=1) as sp,
        ):
            # index tiles: chunk-0's A-columns load on the Pool engine itself
            # (no cross-engine sem hop before the first gather); the rest
            # load in parallel on sync, masks on the scalar engine's HWDGE.
            idx_t = idxp.tile([P, 2 * NBLK + 2 * MBLK], i32)
            nc.gpsimd.dma_start(idx_t[:, 0:BPC], idx.ap()[:, 0:BPC])
            nc.sync.dma_start(idx_t[:, BPC : 2 * BPC], idx.ap()[:, BPC : 2 * BPC])
            nc.sync.dma_start(idx_t[:, 2 * BPC :], idx.ap()[:, 2 * BPC :])
            fm_t = idxp.tile([P, MBLK + BPC], f32)
            nc.scalar.dma_start(fm_t[:], fm.ap())

            def inA_c(c):
                return idx_t[:, 2 * BPC * c : 2 * BPC * c + BPC]

            def inB_c(c):
                return idx_t[:, 2 * BPC * c + BPC : 2 * BPC * (c + 1)]

            imA_t = idx_t[:, 2 * NBLK : 2 * NBLK + MBLK]
            imB_t = idx_t[:, 2 * NBLK + MBLK : 2 * NBLK + 2 * MBLK]
            mmask_t = fm_t[:, 0:MBLK]
            npad_t = fm_t[:, MBLK : MBLK + BPC]

            sums = sp.tile([P, 1 + NCH], f32)
            margin_t = sp.tile([P, 1], f32)
            nc.vector.memset(margin_t[:], MARGIN)

            # --- nonmatch: one batched gather pair per chunk ---
            for c in range(NCH):
                ga = gp.tile([P, BPC * D], f32, tag="ga")
                gather(ga[:], eA, inA_c(c))
                gb = gp.tile([P, BPC * D], f32, tag="gb")
                gather(gb[:], eB, inB_c(c))

                nd = cp.tile([P, BPC * D], f32, tag="nd")
                nc.vector.tensor_sub(nd[:], ga[:], gb[:])
                nsq = cp.tile([P, BPC * D], f32, tag="nsq")
                nc.scalar.square(nsq[:], nd[:])
                dist = cp.tile([P, BPC], f32, tag="dist")
                nc.vector.tensor_reduce(
                    dist[:],
                    nsq[:].rearrange("p (s d) -> p s d", d=D),
                    axis=X,
                    op=ADD,
                )
                if c == NCH - 1:
                    # pad samples: add 1e9 to their distance so the hinge
                    # is exactly 0
                    nc.vector.tensor_add(dist[:], dist[:], npad_t)
                hng = cp.tile([P, BPC], f32, tag="hng")
                nc.scalar.activation(
                    hng[:],
                    dist[:],
                    Relu,
                    bias=margin_t[:],
                    scale=-1.0,
                    accum_out=sums[:, 1 + c : 2 + c],
                )

            # --- match: one batched gather pair for all 20 blocks ---
            ma = gp.tile([P, MBLK * D], f32, tag="ma")
            gather(ma[:], eA, imA_t)
            mb = gp.tile([P, MBLK * D], f32, tag="mb")
            gather(mb[:], eB, imB_t)
            md = cp.tile([P, MBLK * D], f32, tag="md")
            nc.vector.tensor_sub(md[:], ma[:], mb[:])
            # mask the pad samples exactly: mdm = md * mmask (broadcast over D)
            mdm = cp.tile([P, MBLK * D], f32, tag="mdm")
            nc.vector.tensor_tensor(
                out=mdm[:].rearrange("p (s d) -> p s d", d=D),
                in0=md[:].rearrange("p (s d) -> p s d", d=D),
                in1=mmask_t.unsqueeze(2).to_broadcast([P, MBLK, D]),
                op=MULT,
            )
            msq = cp.tile([P, MBLK * D], f32, tag="msq")
            nc.scalar.activation(
                msq[:],
                mdm[:],
                mybir.ActivationFunctionType.Square,
                accum_out=sums[:, 0:1],
            )

            # per-partition sums go straight out; host does the final
            # cross-partition + cross-core reduction (it already sums cores)
            nc.sync.dma_start(out.ap(), sums[:])

    nc.compile()
    return nc


def _get_nc():
    if "nc" not in _CACHE:
        _CACHE["nc"] = _build_nc()
    return _CACHE["nc"]


def _blocked(idx_1d, nblocks):
    """[n] -> [128, nblocks] with sample s at [s % 128, s // 128]; pad with 0."""
    out = np.zeros((P, nblocks), np.int32)
    n = idx_1d.shape[0]
    full = n // P
    out[:, :full] = idx_1d[: full * P].reshape(full, P).T
    rem = n - full * P
    if rem:
        out[:rem, full] = idx_1d[full * P :]
    return out


def _in_maps(outA, outB, matchA, matchB, nonMatchA, nonMatchB):
    outA = np.ascontiguousarray(np.asarray(outA, dtype=np.float32))
    outB = np.ascontiguousarray(np.asarray(outB, dtype=np.float32))
    matchA = np.asarray(matchA).astype(np.int32)
    matchB = np.asarray(matchB).astype(np.int32)
    nonMatchA = np.asarray(nonMatchA).astype(np.int32)
    nonMatchB = np.asarray(nonMatchB).astype(np.int32)

    fm = np.zeros((P, MBLK + BPC), np.float32)
    fm[:, : MBLK - 1] = 1.0
    fm[:M_REM, MBLK - 1] = 1.0
    fm[N_REM:, MBLK + BPC - 1] = 1e9

    maps = []
    for c in range(NCORES):
        b, h = c // 2, c % 2
        bnA = _blocked(nonMatchA[b, h * NH : (h + 1) * NH], NBLK)
        bnB = _blocked(nonMatchB[b, h * NH : (h + 1) * NH], NBLK)
        idx = np.concatenate(
            [
                bnA[:, 0:BPC],
                bnB[:, 0:BPC],
                bnA[:, BPC:NBLK],
                bnB[:, BPC:NBLK],
                _blocked(matchA[b, h * MH : (h + 1) * MH], MBLK),
                _blocked(matchB[b, h * MH : (h + 1) * MH], MBLK),
            ],
            axis=1,
        )
        maps.append(
            {
                "eA": outA[b],
                "eB": outB[b],
                "idx": idx,
                "fm": fm,
            }
        )
    return maps


def kernel(outA, outB, matchA, matchB, nonMatchA, nonMatchB):
    global LAST_EXEC_NS
    from concourse import bass_utils

    nc = _get_nc()
    maps = _in_maps(outA, outB, matchA, matchB, nonMatchA, nonMatchB)

    kwargs = {}
    if os.environ.get("KERNEL_TRACE", "0") == "1":
        kwargs["trace"] = True
    r = bass_utils.run_bass_kernel_spmd(
        nc, maps, core_ids=list(range(NCORES)), **kwargs
    )
    LAST_EXEC_NS = r.exec_time_ns

    partial = np.stack(
        [np.asarray(r.results[c]["out"]) for c in range(NCORES)]
    )  # [8, 128, 1+NCH]
    match_loss = partial[:, :, 0].sum(dtype=np.float64) / M_MATCH
    nonmatch_loss = (
        NON_MATCH_WEIGHT * partial[:, :, 1:].sum(dtype=np.float64) / M_NONMATCH
    )
    contrastive = match_loss + nonmatch_loss
    return (
        np.float32(contrastive),
        np.float32(match_loss),
        np.float32(nonmatch_loss),
    )


# revision 10
# speedup vs baseline: 1.0385x; 1.0385x over previous
"""ContrastiveLoss Trainium2 kernel.

Strategy (data-parallel over 8 NeuronCores):
  - 8 cores = 4 batches x 2 halves. Core c handles batch b=c//2, half h=c%2:
    2500 match pairs + 25000 non-match pairs.
  - Gather primitive: SWDGE vector-indirect DMA (`indirect_dma_start`) with a
    MULTI-COLUMN index AP: idx [128, k] + out [128, k*D] gathers 128*k rows
    (one 64 B row per descriptor) in a single instruction, so the ~1 us
    SWDGE fixed overhead is paid ~6 times per core instead of ~470.
    Samples are column-blocked: sample s = block j * 128 + partition p, so
    block j's indices live in idx[:, j] and its rows land in g[:, 16j:16j+16].
  - Per-sample math on DVE/ACT (overlapped with the gather stream):
      match partial  = sum((mA-mB)^2)              (DVE sub + ACT sq-accum)
      nonmatch partial = sum(relu(0.5-||nA-nB||^2)) (DVE sub, ACT square,
        DVE grouped reduce over D=16, ACT relu with fused accumulate)
  - Padding: tail samples use index 0 on both sides; a host-built {0,1} mask
    zeroes pad match diffs exactly, and a host-built additive bias pushes pad
    nonmatch distances to 1e9 so the hinge is exactly 0.
  - Partition reduction via a ones-vector TensorE matmul into PSUM.
  - Each core outputs [1,2] raw partial sums; the host combines 8x2 scalars
    and applies the 1/5000 and 1/50000 normalizations.

kernel() takes the FULL (unsharded) inputs and returns the full output tuple
(contrastive_loss_sum, match_loss_sum, nonmatch_loss_sum) like the reference.
"""

import os

import numpy as np

# Problem constants (hardcoded per task spec).
B, N, D = 4, 307200, 16
M_MATCH, M_NONMATCH = 5000, 50000
MARGIN = 0.5
NON_MATCH_WEIGHT = 1.0
NCORES = 8

P = 128
MH = M_MATCH // 2  # 2500 match samples per core
NH = M_NONMATCH // 2  # 25000 nonmatch samples per core
MBLK = (MH + P - 1) // P  # 20 match blocks (last one partial: 2500=19*128+68)
NBLK = (NH + P - 1) // P  # 196 nonmatch blocks (25000=195*128+40)
M_REM = MH - (MBLK - 1) * P  # 68 real rows in last match block
N_REM = NH - (NBLK - 1) * P  # 40 real rows in last nonmatch block
NCH = 2  # nonmatch gather chunks (pipeline SWDGE gen with DMA transfer)
BPC = NBLK // NCH  # 98 blocks per chunk
assert NCH * BPC == NBLK

LAST_EXEC_NS = None

_CACHE = {}


def _build_nc():
    import concourse.bacc as bacc
    import concourse.mybir as mybir
    import concourse.tile as tile
    from concourse import bass

    f32 = mybir.dt.float32
    i32 = mybir.dt.int32
    X = mybir.AxisListType.X
    ADD = mybir.AluOpType.add
    MULT = mybir.AluOpType.mult
    Relu = mybir.ActivationFunctionType.Relu

    nc = bacc.Bacc("TRN2", target_bir_lowering=False, debug=False)
    eA = nc.dram_tensor("eA", (N, D), f32, kind="ExternalInput")
    eB = nc.dram_tensor("eB", (N, D), f32, kind="ExternalInput")
    # all index columns in one tensor: [inA | inB | imA | imB]
    idx = nc.dram_tensor("idx", (P, 2 * NBLK + 2 * MBLK), i32, kind="ExternalInput")
    # pad handling in one tensor: [mmask (MBLK) | npad (BPC)].
    # mmask is 1.0 for real match samples else 0.0; npad adds 1e9 to pad
    # nonmatch distances (hinge -> exactly 0)
    fm = nc.dram_tensor("fm", (P, MBLK + BPC), f32, kind="ExternalInput")
    out = nc.dram_tensor("out", (P, 1 + 2 * NCH), f32, kind="ExternalOutput")

    def gather(dst_ap, src, idx_ap):
        nc.gpsimd.indirect_dma_start(
            out=dst_ap,
            out_offset=None,
            in_=src.ap(),
            in_offset=bass.IndirectOffsetOnAxis(ap=idx_ap, axis=0),
        )

    with tile.TileContext(nc) as tc:
        with (
            tc.tile_pool(name="idx", bufs=1) as idxp,
            tc.tile_pool(name="gath", bufs=2) as gp,
            tc.tile_pool(name="cmp", bufs=2) as cp,
            tc.tile_pool(name="sums", bufs=1) as sp,
        ):
            # index tiles: chunk-0 columns land first so its gathers start
            # early; masks load in parallel on the scalar engine's HWDGE.
            idx_t = idxp.tile([P, 2 * NBLK + 2 * MBLK], i32)
            nc.sync.dma_start(idx_t[:, 0 : 2 * BPC], idx.ap()[:, 0 : 2 * BPC])
            nc.sync.dma_start(idx_t[:, 2 * BPC :], idx.ap()[:, 2 * BPC :])
            fm_t = idxp.tile([P, MBLK + BPC], f32)
            nc.scalar.dma_start(fm_t[:], fm.ap())

            def inA_c(c):
                return idx_t[:, 2 * BPC * c : 2 * BPC * c + BPC]

            def inB_c(c):
                return idx_t[:, 2 * BPC * c + BPC : 2 * BPC * (c + 1)]

            imA_t = idx_t[:, 2 * NBLK : 2 * NBLK + MBLK]
            imB_t = idx_t[:, 2 * NBLK + MBLK : 2 * NBLK + 2 * MBLK]
            mmask_t = fm_t[:, 0:MBLK]
            npad_t = fm_t[:, MBLK : MBLK + BPC]

            sums = sp.tile([P, 1 + 2 * NCH], f32)
            margin_t = sp.tile([P, 1], f32)
            nc.vector.memset(margin_t[:], MARGIN)

            # --- nonmatch: one batched gather pair per chunk ---
            for c in range(NCH):
                ga = gp.tile([P, BPC * D], f32, tag="ga")
                gather(ga[:], eA, inA_c(c))
                gb = gp.tile([P, BPC * D], f32, tag="gb")
                gather(gb[:], eB, inB_c(c))

                nd = cp.tile([P, BPC * D], f32, tag="nd")
                nsq = cp.tile([P, BPC * D], f32, tag="nsq")
                dist = cp.tile([P, BPC], f32, tag="dist")
                hng = cp.tile([P, BPC], f32, tag="hng")
                HB = BPC // 2
                for hf in range(2):
                    lo, hi = hf * HB, (hf + 1) * HB
                    nc.vector.tensor_sub(
                        nd[:, lo * D : hi * D],
                        ga[:, lo * D : hi * D],
                        gb[:, lo * D : hi * D],
                    )
                    nc.scalar.square(
                        nsq[:, lo * D : hi * D], nd[:, lo * D : hi * D]
                    )
                    nc.vector.tensor_reduce(
                        dist[:, lo:hi],
                        nsq[:, lo * D : hi * D].rearrange(
                            "p (s d) -> p s d", d=D
                        ),
                        axis=X,
                        op=ADD,
                    )
                    if c == NCH - 1:
                        # pad samples: add 1e9 to their distance so the
                        # hinge is exactly 0
                        nc.vector.tensor_add(
                            dist[:, lo:hi], dist[:, lo:hi], npad_t[:, lo:hi]
                        )
                    col = 1 + 2 * c + hf
                    nc.scalar.activation(
                        hng[:, lo:hi],
                        dist[:, lo:hi],
                        Relu,
                        bias=margin_t[:],
                        scale=-1.0,
                        accum_out=sums[:, col : col + 1],
                    )

            # --- match: one batched gather pair for all 20 blocks ---
            ma = gp.tile([P, MBLK * D], f32, tag="ma")
            gather(ma[:], eA, imA_t)
            mb = gp.tile([P, MBLK * D], f32, tag="mb")
            gather(mb[:], eB, imB_t)
            md = cp.tile([P, MBLK * D], f32, tag="md")
            nc.vector.tensor_sub(md[:], ma[:], mb[:])
            # mask the pad samples exactly: mdm = md * mmask (broadcast over D)
            mdm = cp.tile([P, MBLK * D], f32, tag="mdm")
            nc.vector.tensor_tensor(
                out=mdm[:].rearrange("p (s d) -> p s d", d=D),
                in0=md[:].rearrange("p (s d) -> p s d", d=D),
                in1=mmask_t.unsqueeze(2).to_broadcast([P, MBLK, D]),
                op=MULT,
            )
            msq = cp.tile([P, MBLK * D], f32, tag="msq")
            nc.scalar.activation(
                msq[:],
                mdm[:],
                mybir.ActivationFunctionType.Square,
                accum_out=sums[:, 0:1],
            )

            # per-partition sums go straight out; host does the final
            # cross-partition + cross-core reduction (it already sums cores)
            nc.sync.dma_start(out.ap(), sums[:])

    nc.compile()
    return nc


def _get_nc():
    if "nc" not in _CACHE:
        _CACHE["nc"] = _build_nc()
    return _CACHE["nc"]


def _blocked(idx_1d, nblocks):
    """[n] -> [128, nblocks] with sample s at [s % 128, s // 128]; pad with 0."""
    out = np.zeros((P, nblocks), np.int32)
    n = idx_1d.shape[0]
    full = n // P
    out[:, :full] = idx_1d[: full * P].reshape(full, P).T
    rem = n - full * P
    if rem:
        out[:rem, full] = idx_1d[full * P :]
    return out


def _in_maps(outA, outB, matchA, matchB, nonMatchA, nonMatchB):
    outA = np.ascontiguousarray(np.asarray(outA, dtype=np.float32))
    outB = np.ascontiguousarray(np.asarray(outB, dtype=np.float32))
    matchA = np.asarray(matchA).astype(np.int32)
    matchB = np.asarray(matchB).astype(np.int32)
    nonMatchA = np.asarray(nonMatchA).astype(np.int32)
    nonMatchB = np.asarray(nonMatchB).astype(np.int32)

    fm = np.zeros((P, MBLK + BPC), np.float32)
    fm[:, : MBLK - 1] = 1.0
    fm[:M_REM, MBLK - 1] = 1.0
    fm[N_REM:, MBLK + BPC - 1] = 1e9

    maps = []
    for c in range(NCORES):
        b, h = c // 2, c % 2
        bnA = _blocked(nonMatchA[b, h * NH : (h + 1) * NH], NBLK)
        bnB = _blocked(nonMatchB[b, h * NH : (h + 1) * NH], NBLK)
        idx = np.concatenate(
            [
                bnA[:, 0:BPC],
                bnB[:, 0:BPC],
                bnA[:, BPC:NBLK],
                bnB[:, BPC:NBLK],
                _blocked(matchA[b, h * MH : (h + 1) * MH], MBLK),
                _blocked(matchB[b, h * MH : (h + 1) * MH], MBLK),
            ],
            axis=1,
        )
        maps.append(
            {
                "eA": outA[b],
                "eB": outB[b],
                "idx": idx,
                "fm": fm,
            }
        )
    return maps


def kernel(outA, outB, matchA, matchB, nonMatchA, nonMatchB):
    global LAST_EXEC_NS
    from concourse import bass_utils

    nc = _get_nc()
    maps = _in_maps(outA, outB, matchA, matchB, nonMatchA, nonMatchB)

    kwargs = {}
    if os.environ.get("KERNEL_TRACE", "0") == "1":
        kwargs["trace"] = True
    r = bass_utils.run_bass_kernel_spmd(
        nc, maps, core_ids=list(range(NCORES)), **kwargs
    )
    LAST_EXEC_NS = r.exec_time_ns

    partial = np.stack(
        [np.asarray(r.results[c]["out"]) for c in range(NCORES)]
    )  # [8, 128, 1+NCH]
    match_loss = partial[:, :, 0].sum(dtype=np.float64) / M_MATCH
    nonmatch_loss = (
        NON_MATCH_WEIGHT * partial[:, :, 1:].sum(dtype=np.float64) / M_NONMATCH
    )
    contrastive = match_loss + nonmatch_loss
    return (
        np.float32(contrastive),
        np.float32(match_loss),
        np.float32(nonmatch_loss),
    )
